# revision 4
# baseline (speedup 1.0000x reference)
"""EqProp free-phase relaxation (value network) on 8 trn2 NeuronCores.

Strategy (tensor parallel over the hidden dim, per sharding hint):
  - core j owns hidden slice j*1024:(j+1)*1024 of W1-columns / W2-rows
  - W2 slice kept SBUF-resident in BOTH layouts (natural + transposed,
    transposed prepared on host) so both per-iteration matvecs run on the
    PE with the tiny vector as the stationary operand (lhsT [128,1]).
  - o (size 2048) is replicated; each iteration the o-gradient partial
    sums rho(h_j) @ W2_j are AllGather-ed and summed locally.
  - state layouts: h-side [128, 8] with h[p*8+a] at [p,a];
    o-side [128, 16] with o[p*16+b] at [p,b]. The matvec contractions are
    grouped to match (k = p*K + sub), so lhsT vector blocks are plain
    column slices of the state tiles.

Iteration recurrence (matches reference.py exactly):
  gh = h - mask_h*(b_h + xW1 + W2 @ rho(o))
  go = o - mask_o*(b_o + rho(h) @ W2)
  adam updates on (h, mh, vh) and (o, mo, vo) with bias correction.
"""

import sys

if "/opt/trn_rl_repo" not in sys.path:
    sys.path.insert(0, "/opt/trn_rl_repo")

import numpy as np

import concourse.bass as bass
import concourse.bacc as bacc
import concourse.tile as tile
import concourse.mybir as mybir
from concourse import bass_utils

F32 = mybir.dt.float32
F32R = mybir.dt.float32r
AO = mybir.AluOpType
AF = mybir.ActivationFunctionType

N_CORES = 8
P = 128
IN_SIZE, HID, OUT = 4096, 8192, 2048
HID_L = HID // N_CORES          # 1024 per-core hidden slice
NA = HID_L // P                 # 8  h sub-blocks
NB = OUT // P                   # 16 o sub-blocks
NC_X = IN_SIZE // P             # 32 x sub-blocks
LR, B1, B2, EPS = 0.01, 0.9, 0.999, 1e-8

# experiment knobs
CFG = {
    "w2_f32r": False,    # relaxed-precision matmul for the W2 matvecs
    "w1_f32r": False,   # W1 one-time matvec precision
}

_BUILD_CACHE = {}


def _build(num_iterations: int):
    key = (num_iterations, CFG["w2_f32r"], CFG["w1_f32r"])
    if key in _BUILD_CACHE:
        return _BUILD_CACHE[key]

    nc = bacc.Bacc("TRN2", target_bir_lowering=False, debug=False,
                   num_devices=N_CORES)

    x_d = nc.dram_tensor("x", [P, NC_X], F32, kind="ExternalInput")
    w1_d = nc.dram_tensor("w1", [P, NC_X, HID_L], F32, kind="ExternalInput")
    w2_d = nc.dram_tensor("w2", [P, NA, OUT], F32, kind="ExternalInput")
    w2t_d = nc.dram_tensor("w2t", [P, NB, HID_L], F32, kind="ExternalInput")
    bh_d = nc.dram_tensor("bh", [P, NA], F32, kind="ExternalInput")
    bo_d = nc.dram_tensor("bo", [P, NB], F32, kind="ExternalInput")
    o_out = nc.dram_tensor("oout", [P, NB], F32, kind="ExternalOutput")

    def mmcast(ap, relaxed):
        return ap.bitcast(F32R) if relaxed else ap

    with tile.TileContext(nc) as tc:
        with (
            tc.tile_pool(name="wpool", bufs=1) as wpool,      # persistent weights
            tc.tile_pool(name="w1pool", bufs=4) as w1pool,    # streamed W1 tiles
            tc.tile_pool(name="spool", bufs=3) as spool,      # iteration state
            tc.tile_pool(name="tpool", bufs=3) as tpool,      # temps
            tc.tile_pool(name="pxp", bufs=1, space="PSUM") as pxp,
            tc.tile_pool(name="pap", bufs=1, space="PSUM") as pap,
            tc.tile_pool(name="pbp", bufs=1, space="PSUM") as pbp,
            tc.tile_pool(name="dram", bufs=3, space="DRAM") as dram,
        ):
            # ---------------- startup: constants + W1 phase ----------------
            xr = wpool.tile([P, NC_X], F32, tag="xr")
            nc.sync.dma_start(xr[:], x_d.ap())
            rx = wpool.tile([P, NC_X], F32, tag="rx")
            nc.vector.tensor_scalar(rx[:], xr[:], 0.0, 1.0, op0=AO.max, op1=AO.min)

            bh_t = wpool.tile([P, NA], F32, tag="bh")
            nc.sync.dma_start(bh_t[:], bh_d.ap())
            bo_t = wpool.tile([P, NB], F32, tag="bo")
            nc.sync.dma_start(bo_t[:], bo_d.ap())

            # xW1 accumulation: psum [1, 1024] as two banks of 512
            px = [pxp.tile([1, 512], F32, tag=f"px{i}", name=f"px{i}") for i in range(2)]
            w1r = CFG["w1_f32r"]
            for c in range(NC_X):
                w1t = w1pool.tile([P, HID_L], F32, tag="w1t", name="w1t")
                nc.sync.dma_start(w1t[:], w1_d.ap()[:, c, :])
                for half in range(2):
                    nc.tensor.matmul(
                        px[half][:],
                        mmcast(rx[:, c : c + 1], w1r),
                        mmcast(w1t[:, half * 512 : (half + 1) * 512], w1r),
                        start=(c == 0),
                        stop=(c == NC_X - 1),
                    )
            # xW1 psum -> sbuf staging -> reshape to [128, 8] h-layout
            stage_x = wpool.tile([1, HID_L], F32, tag="stage_x")
            for half in range(2):
                nc.scalar.copy(stage_x[:, half * 512 : (half + 1) * 512], px[half][:])
            xw1 = wpool.tile([P, NA], F32, tag="xw1")
            nc.sync.dma_start(xw1[:], stage_x[:])
            # ch = b_h + xW1  (constant across iterations)
            ch = wpool.tile([P, NA], F32, tag="ch")
            nc.vector.tensor_add(ch[:], bh_t[:], xw1[:])

            # persistent W2 slabs (natural + transposed)
            w2sl = []
            for a in range(NA):
                t = wpool.tile([P, OUT], F32, tag=f"w2_{a}", name=f"w2_{a}")
                nc.sync.dma_start(t[:], w2_d.ap()[:, a, :])
                w2sl.append(t)
            w2tsl = []
            for b in range(NB):
                t = wpool.tile([P, HID_L], F32, tag=f"w2t_{b}", name=f"w2t_{b}")
                nc.sync.dma_start(t[:], w2t_d.ap()[:, b, :])
                w2tsl.append(t)

            # ---------------- state init ----------------
            h = spool.tile([P, NA], F32, tag="h")
            o = spool.tile([P, NB], F32, tag="o")
            mh = spool.tile([P, NA], F32, tag="mh")
            vh = spool.tile([P, NA], F32, tag="vh")
            mo = spool.tile([P, NB], F32, tag="mo")
            vo = spool.tile([P, NB], F32, tag="vo")
            for t_ in (h, o, mh, vh, mo, vo):
                nc.vector.memset(t_[:], 0.0)
            # pre-scaled first/second moment carries (B1*m, B2*v)
            mhB = spool.tile([P, NA], F32, tag="mhB")
            vhB = spool.tile([P, NA], F32, tag="vhB")
            moB = spool.tile([P, NB], F32, tag="moB")
            voB = spool.tile([P, NB], F32, tag="voB")
            for t_ in (mhB, vhB, moB, voB):
                nc.vector.memset(t_[:], 0.0)

            rho_o = spool.tile([P, NB], F32, tag="rho_o")
            mask_o = spool.tile([P, NB], F32, tag="mask_o")
            nc.vector.memset(rho_o[:], 0.0)
            nc.vector.memset(mask_o[:], 1.0)

            zero_u = wpool.tile([P, NA], F32, tag="zero_u")
            nc.vector.memset(zero_u[:], 0.0)
            zero_s = wpool.tile([P, NB], F32, tag="zero_s")
            nc.vector.memset(zero_s[:], 0.0)

            pa = [pap.tile([1, 512], F32, tag=f"pa{i}", name=f"pa{i}") for i in range(4)]
            pb = [pbp.tile([1, 512], F32, tag=f"pb{i}", name=f"pb{i}") for i in range(2)]

            w2r = CFG["w2_f32r"]

            def adam_update(gname, g, p_t, mB, vB, sc1, sc2, shape_n, newtags):
                """One adam step. Returns (p_new, m_new, v_new, mB_new, vB_new)."""
                m_n = spool.tile([P, shape_n], F32, tag=newtags[0])
                nc.vector.scalar_tensor_tensor(
                    m_n[:], g[:], 1.0 - B1, mB[:], op0=AO.mult, op1=AO.add)
                gg = tpool.tile([P, shape_n], F32, tag=f"gg_{gname}")
                nc.vector.tensor_mul(gg[:], g[:], g[:])
                v_n = spool.tile([P, shape_n], F32, tag=newtags[1])
                nc.vector.scalar_tensor_tensor(
                    v_n[:], gg[:], 1.0 - B2, vB[:], op0=AO.mult, op1=AO.add)
                sq = tpool.tile([P, shape_n], F32, tag=f"sq_{gname}")
                nc.scalar.activation(sq[:], v_n[:], AF.Sqrt, scale=float(sc2))
                d = tpool.tile([P, shape_n], F32, tag=f"d_{gname}")
                nc.vector.tensor_scalar_add(d[:], sq[:], EPS)
                r = tpool.tile([P, shape_n], F32, tag=f"r_{gname}")
                nc.vector.reciprocal(r[:], d[:])
                upd = tpool.tile([P, shape_n], F32, tag=f"upd_{gname}")
                nc.vector.scalar_tensor_tensor(
                    upd[:], m_n[:], float(sc1), r[:], op0=AO.mult, op1=AO.mult)
                p_n = spool.tile([P, shape_n], F32, tag=newtags[2])
                nc.vector.tensor_sub(p_n[:], p_t[:], upd[:])
                # pre-scaled carries for next iteration (off critical path)
                mB_n = spool.tile([P, shape_n], F32, tag=newtags[3])
                nc.vector.tensor_scalar_mul(mB_n[:], m_n[:], B1)
                vB_n = spool.tile([P, shape_n], F32, tag=newtags[4])
                nc.vector.tensor_scalar_mul(vB_n[:], v_n[:], B2)
                return p_n, m_n, v_n, mB_n, vB_n

            for t in range(1, num_iterations + 1):
                # bias-corrected step-size constants for this iteration
                sc1 = LR / (1.0 - B1 ** t)   # multiplies m
                sc2 = 1.0 / (1.0 - B2 ** t)  # scales v inside sqrt

                # ---- A path: partial = rho(h) @ W2_j  -> AllGather -> s ----
                rho_h = spool.tile([P, NA], F32, tag="rho_h")
                nc.vector.tensor_scalar(rho_h[:], h[:], 0.0, 1.0,
                                        op0=AO.max, op1=AO.min)
                mask_h = spool.tile([P, NA], F32, tag="mask_h")
                nc.vector.tensor_tensor(mask_h[:], h[:], rho_h[:], op=AO.is_equal)

                if t >= 2:
                    stage_o = tpool.tile([1, OUT], F32, tag="stage_o")
                    for nt in range(4):
                        for a in range(NA):
                            nc.tensor.matmul(
                                pa[nt][:],
                                mmcast(rho_h[:, a : a + 1], w2r),
                                mmcast(w2sl[a][:, nt * 512 : (nt + 1) * 512], w2r),
                                start=(a == 0),
                                stop=(a == NA - 1),
                            )
                        nc.scalar.copy(
                            stage_o[:, nt * 512 : (nt + 1) * 512], pa[nt][:])
                    cc_in = dram.tile([OUT], F32, tag="cc_in")
                    nc.sync.dma_start(cc_in[:], stage_o[:])
                    cc_out = dram.tile([N_CORES, OUT], F32, tag="cc_out")
                    nc.gpsimd.collective_compute(
                        "AllGather",
                        AO.bypass,
                        replica_groups=[list(range(N_CORES))],
                        ins=[cc_in[:]],
                        outs=[cc_out[:]],
                    )
                    g_sb = tpool.tile([P, N_CORES, NB], F32, tag="g_sb")
                    nc.sync.dma_start(
                        g_sb[:], cc_out[:].rearrange("r (p b) -> p r b", p=P))
                    t1 = tpool.tile([P, 4, NB], F32, tag="t1")
                    nc.vector.tensor_add(t1[:], g_sb[:, 0:4, :], g_sb[:, 4:8, :])
                    t2 = tpool.tile([P, 2, NB], F32, tag="t2")
                    nc.vector.tensor_add(t2[:], t1[:, 0:2, :], t1[:, 2:4, :])
                    svec = tpool.tile([P, NB], F32, tag="svec")
                    nc.vector.tensor_add(svec[:], t2[:, 0, :], t2[:, 1, :])
                else:
                    svec = zero_s  # rho(h_1) = 0 exactly

                # ---- o chain ----
                q = tpool.tile([P, NB], F32, tag="q")
                nc.vector.tensor_add(q[:], svec[:], bo_t[:])
                q2 = tpool.tile([P, NB], F32, tag="q2")
                nc.vector.tensor_mul(q2[:], mask_o[:], q[:])
                go = tpool.tile([P, NB], F32, tag="go")
                nc.vector.tensor_sub(go[:], o[:], q2[:])
                o_n, mo_n, vo_n, moB_n, voB_n = adam_update(
                    "o", go, o, moB, voB, sc1, sc2, NB,
                    ("mo", "vo", "o", "moB", "voB"))
                rho_o_n = spool.tile([P, NB], F32, tag="rho_o")
                nc.vector.tensor_scalar(rho_o_n[:], o_n[:], 0.0, 1.0,
                                        op0=AO.max, op1=AO.min)
                mask_o_n = spool.tile([P, NB], F32, tag="mask_o")
                nc.vector.tensor_tensor(mask_o_n[:], o_n[:], rho_o_n[:],
                                        op=AO.is_equal)

                # ---- B path: u = W2_j @ rho(o) (uses rho_o of THIS iter's o) ----
                if t >= 2:
                    stage_u = tpool.tile([1, HID_L], F32, tag="stage_u")
                    for it in range(2):
                        for b in range(NB):
                            nc.tensor.matmul(
                                pb[it][:],
                                mmcast(rho_o[:, b : b + 1], w2r),
                                mmcast(w2tsl[b][:, it * 512 : (it + 1) * 512], w2r),
                                start=(b == 0),
                                stop=(b == NB - 1),
                            )
                        nc.scalar.copy(
                            stage_u[:, it * 512 : (it + 1) * 512], pb[it][:])
                    usb = tpool.tile([P, NA], F32, tag="usb")
                    nc.sync.dma_start(usb[:], stage_u[:])
                else:
                    usb = zero_u  # rho(o_1) = 0 exactly

                # ---- h chain ----
                qh = tpool.tile([P, NA], F32, tag="qh")
                nc.vector.tensor_add(qh[:], usb[:], ch[:])
                qh2 = tpool.tile([P, NA], F32, tag="qh2")
                nc.vector.tensor_mul(qh2[:], mask_h[:], qh[:])
                gh = tpool.tile([P, NA], F32, tag="gh")
                nc.vector.tensor_sub(gh[:], h[:], qh2[:])
                h_n, mh_n, vh_n, mhB_n, vhB_n = adam_update(
                    "h", gh, h, mhB, vhB, sc1, sc2, NA,
                    ("mh", "vh", "h", "mhB", "vhB"))

                h, mh, vh, mhB, vhB = h_n, mh_n, vh_n, mhB_n, vhB_n
                o, mo, vo, moB, voB = o_n, mo_n, vo_n, moB_n, voB_n
                rho_o, mask_o = rho_o_n, mask_o_n

            nc.sync.dma_start(o_out.ap(), o[:])

    nc.compile()
    _BUILD_CACHE[key] = nc
    return nc


def _make_in_maps(x, W1, W2, b_h, b_o):
    x = np.ascontiguousarray(x, dtype=np.float32)
    W1 = np.ascontiguousarray(W1, dtype=np.float32)
    W2 = np.ascontiguousarray(W2, dtype=np.float32)
    b_h = np.ascontiguousarray(b_h, dtype=np.float32)
    b_o = np.ascontiguousarray(b_o, dtype=np.float32)

    xr = x.reshape(P, NC_X)
    bor = b_o.reshape(P, NB)
    in_maps = []
    for j in range(N_CORES):
        sl = slice(j * HID_L, (j + 1) * HID_L)
        w2s = W2[sl, :]
        in_maps.append({
            "x": xr,
            "w1": np.ascontiguousarray(W1[:, sl]).reshape(P, NC_X, HID_L),
            "w2": np.ascontiguousarray(w2s).reshape(P, NA, OUT),
            "w2t": np.ascontiguousarray(w2s.T).reshape(P, NB, HID_L),
            "bh": b_h[sl].reshape(P, NA),
            "bo": bor,
        })
    return in_maps


def run(x, W1, W2, b_h, b_o, num_iterations, trace=False, **trace_kwargs):
    nc = _build(int(num_iterations))
    in_maps = _make_in_maps(x, W1, W2, b_h, b_o)
    res = bass_utils.run_bass_kernel_spmd(
        nc, in_maps, core_ids=list(range(N_CORES)), trace=trace, **trace_kwargs)
    out = res.results[0]["oout"].reshape(OUT).astype(np.float32)
    return out, res


def kernel(x, W1, W2, b_h, b_o, num_iterations):
    out, _ = run(x, W1, W2, b_h, b_o, num_iterations)
    return out
